# revision 14
# baseline (speedup 1.0000x reference)
"""DeepSeek-V2 MoE gate (group-limited greedy top-k routing) on 8 trn2 NeuronCores.

Reference computation (per token t over E=160 experts in G=8 groups of 20):
    logits = x @ W^T                       [T, E]
    scores = softmax(logits)
    group_scores[g] = max over group g of scores
    keep top-3 groups; mask scores of other groups to 0
    topk_weight, topk_idx = top_k(masked scores, 6); topk_weight *= 16.0

Sharding: tokens (B*S = 16384) split evenly across the 8 cores; the small
[160, 5120] gate weight is replicated (pre-arranged host-side).

Kernel layout trick: the tensor engine contracts over the partition axis, so
both matmul operands need hidden (H=5120) on partitions. hidden_states is
[T, H] row-major in DRAM; a plain transposed load would be descriptor-tiny.
Instead partition p holds the 40 consecutive hidden values h = p*40 + j
(j = 0..39), so each token's DMA is its fully contiguous 20KB row landing as
one 160B run per partition, and the contraction runs j = 0..39 as 40 k-tiles
with stride-40 stationary APs. The weight is pre-permuted on host to match:
w_sb[p, j*160 + e] = W[e, p*40 + j].

Selection runs on raw logits (softmax is monotonic; the top-3-group test by
max-score equals the test by max-logit), so only the final 6 weights and the
softmax denominator need exp().
"""

import numpy as np

import concourse.bacc as bacc
import concourse.mybir as mybir
from concourse import bass_utils
from concourse.tile import TileContext

# Problem constants (hardcoded per the harness contract).
B, S, H = 4, 4096, 5120
E = 160                 # experts
G = 8                   # groups
EG = E // G             # experts per group (20)
TOP_K = 6
TOPK_GROUP = 3
ROUTED_SCALING = 16.0
N_CORES = 8
T_TOTAL = B * S         # 16384
T_CORE = T_TOTAL // N_CORES  # 2048
P = 128                 # SBUF partitions
J = H // P              # hidden values per partition (40) = number of k-tiles
NEG_BIG = -1.0e30

F32 = mybir.dt.float32
F32R = mybir.dt.float32r  # fp32 with 17-bit mantissa; PE streams it 4x faster
BF16 = mybir.dt.bfloat16
U32 = mybir.dt.uint32
ALU = mybir.AluOpType
ACTF = mybir.ActivationFunctionType
AX = mybir.AxisListType


def emit_gate(tc, x_ap, w_ap, oi_ap, ow_ap):
    """Emit the gate kernel body into TileContext `tc`.

    x_ap:  [T, H] f32 DRAM (T % 128 == 0)
    w_ap:  [P, J*E] f32 DRAM (pre-permuted weight, see module docstring)
    oi_ap: [T, TOP_K] u32 DRAM out (expert indices)
    ow_ap: [T, TOP_K] f32 DRAM out (routing weights)
    """
    nc = tc.nc
    T = x_ap.shape[0]
    assert T % P == 0
    n_tiles = T // P

    with (
        tc.tile_pool(name="wpool", bufs=1) as wpool,
        tc.tile_pool(name="xpool", bufs=3) as xpool,
        tc.tile_pool(name="psum", bufs=4, space="PSUM") as psum_pool,
        tc.tile_pool(name="small", bufs=6) as small,
        tc.tile_pool(name="bigt", bufs=3) as bigt,
    ):
        w_sb = wpool.tile([P, J * E], F32)
        nc.sync.dma_start(w_sb[:], w_ap)

        for tt in range(n_tiles):
            # x tile: [p, t*J + j] = x[t0 + t, p*J + j]
            xt = xpool.tile([P, P * J], F32)
            src = x_ap[tt * P : (tt + 1) * P, :].rearrange("t (p j) -> p t j", p=P)
            nc.sync.dma_start(xt[:].rearrange("p (t j) -> p t j", j=J), src)
            xt3 = xt[:].rearrange("p (t j) -> p t j", j=J)

            # logits[t, e] accumulated over the 40 k-tiles
            ps = psum_pool.tile([P, E], F32)
            for j in range(J):
                nc.tensor.matmul(
                    ps[:],
                    xt3[:, :, j],                  # stationary [128h, 128t]
                    w_sb[:, j * E : (j + 1) * E],  # moving     [128h, 160e]
                    start=(j == 0),
                    stop=(j == J - 1),
                )

            ps3 = ps[:].rearrange("p (g i) -> p g i", i=EG)

            # group max of logits -> top-3-group additive penalty mask
            gmax = small.tile([P, G], F32)
            nc.vector.tensor_reduce(gmax[:], ps3, axis=AX.X, op=ALU.max)
            gsort = small.tile([P, 8], F32)
            nc.vector.max(gsort[:], gmax[:])
            gpen = small.tile([P, G], F32)  # 0 for kept groups, NEG_BIG for dropped
            nc.vector.tensor_scalar(
                gpen[:], gmax[:], gsort[:, TOPK_GROUP - 1 : TOPK_GROUP], NEG_BIG,
                op0=ALU.is_lt, op1=ALU.mult,
            )

            # masked logits = logits + penalty(group)
            masked = bigt.tile([P, E], F32)
            nc.vector.scalar_tensor_tensor(
                masked[:].rearrange("p (g i) -> p g i", i=EG),
                ps3,
                1.0,
                gpen[:, :, None].to_broadcast((P, G, EG)),
                op0=ALU.mult,
                op1=ALU.add,
            )

            # top-8 masked logits (descending) + expert indices
            v8 = small.tile([P, 8], F32)
            nc.vector.max(v8[:], masked[:])
            i8 = small.tile([P, 8], U32)
            nc.vector.max_index(i8[:], v8[:], masked[:])

            # softmax pieces: global max logit is v8[:,0] (the best group holds it)
            nrmax = small.tile([P, 1], F32)
            nc.vector.tensor_scalar_mul(nrmax[:], v8[:, 0:1], -1.0)
            exps = bigt.tile([P, E], F32)
            ssum = small.tile([P, 1], F32)
            nc.scalar.activation(
                exps[:], ps[:], ACTF.Exp, bias=nrmax[:], scale=1.0, accum_out=ssum[:]
            )
            rcp = small.tile([P, 1], F32)
            nc.vector.reciprocal(rcp[:], ssum[:])
            scl = small.tile([P, 1], F32)
            nc.vector.tensor_scalar_mul(scl[:], rcp[:], ROUTED_SCALING)

            # weights = exp(v6 - rmax) * 16 / ssum
            e6 = small.tile([P, TOP_K], F32)
            nc.scalar.activation(e6[:], v8[:, 0:TOP_K], ACTF.Exp, bias=nrmax[:], scale=1.0)
            w6 = small.tile([P, TOP_K], F32)
            nc.vector.tensor_scalar_mul(w6[:], e6[:], scl[:])

            nc.sync.dma_start(oi_ap[tt * P : (tt + 1) * P, :], i8[:, 0:TOP_K])
            nc.sync.dma_start(ow_ap[tt * P : (tt + 1) * P, :], w6[:])


def emit_gate_hilo(tc, x_ap, whi_ap, wlo_ap, oi_ap, ow_ap, terms=3):
    """Split-precision gate: x and W decomposed as bf16 hi + lo; logits =
    hi@Whi + hi@Wlo + lo@Whi (+ lo@Wlo with terms=4) accumulated in fp32
    PSUM (error ~2^-18). bf16 matmuls run ~4x faster than fp32 on the PE.
    W's split is precomputed on host; x's is done on-chip (ACT casts hi,
    DVE computes lo = x - hi)."""
    nc = tc.nc
    T = x_ap.shape[0]
    assert T % P == 0
    n_tiles = T // P

    with (
        tc.tile_pool(name="wpool", bufs=1) as wpool,
        tc.tile_pool(name="xpool", bufs=3) as xpool,
        tc.tile_pool(name="hpool", bufs=3) as hpool,
        tc.tile_pool(name="lpool", bufs=3) as lpool,
        tc.tile_pool(name="psum", bufs=4, space="PSUM") as psum_pool,
        tc.tile_pool(name="small", bufs=6) as small,
        tc.tile_pool(name="bigt", bufs=3) as bigt,
    ):
        whi_sb = wpool.tile([P, J * E], BF16)
        nc.sync.dma_start(whi_sb[:], whi_ap)
        wlo_sb = wpool.tile([P, J * E], BF16)
        nc.sync.dma_start(wlo_sb[:], wlo_ap)

        for tt in range(n_tiles):
            xt = xpool.tile([P, P * J], F32)
            src = x_ap[tt * P : (tt + 1) * P, :].rearrange("t (p j) -> p t j", p=P)
            nc.sync.dma_start(xt[:].rearrange("p (t j) -> p t j", j=J), src)

            hi = hpool.tile([P, P * J], BF16)
            nc.scalar.copy(hi[:], xt[:])
            lo = lpool.tile([P, P * J], BF16)
            nc.vector.scalar_tensor_tensor(
                lo[:], xt[:], 1.0, hi[:], op0=ALU.mult, op1=ALU.subtract
            )
            hi3 = hi[:].rearrange("p (t j) -> p t j", j=J)
            lo3 = lo[:].rearrange("p (t j) -> p t j", j=J)

            ps = psum_pool.tile([P, E], F32)
            for j in range(J):
                wsl = slice(j * E, (j + 1) * E)
                nc.tensor.matmul(
                    ps[:], hi3[:, :, j], whi_sb[:, wsl], start=(j == 0), stop=False
                )
                nc.tensor.matmul(
                    ps[:], hi3[:, :, j], wlo_sb[:, wsl], start=False, stop=False
                )
                nc.tensor.matmul(
                    ps[:], lo3[:, :, j], whi_sb[:, wsl],
                    start=False, stop=(terms == 3 and j == J - 1),
                )
                if terms == 4:
                    nc.tensor.matmul(
                        ps[:], lo3[:, :, j], wlo_sb[:, wsl],
                        start=False, stop=(j == J - 1),
                    )

            _emit_epilogue(tc, small, bigt, ps, oi_ap, ow_ap, tt)


def _emit_epilogue(tc, small, bigt, ps, oi_ap, ow_ap, tt):
    nc = tc.nc
    ps3 = ps[:].rearrange("p (g i) -> p g i", i=EG)
    gmax = small.tile([P, G], F32)
    nc.vector.tensor_reduce(gmax[:], ps3, axis=AX.X, op=ALU.max)
    gsort = small.tile([P, 8], F32)
    nc.vector.max(gsort[:], gmax[:])
    gpen = small.tile([P, G], F32)
    nc.vector.tensor_scalar(
        gpen[:], gmax[:], gsort[:, TOPK_GROUP - 1 : TOPK_GROUP], NEG_BIG,
        op0=ALU.is_lt, op1=ALU.mult,
    )
    masked = bigt.tile([P, E], F32)
    nc.vector.scalar_tensor_tensor(
        masked[:].rearrange("p (g i) -> p g i", i=EG),
        ps3, 1.0,
        gpen[:, :, None].to_broadcast((P, G, EG)),
        op0=ALU.mult, op1=ALU.add,
    )
    v8 = small.tile([P, 8], F32)
    nc.vector.max(v8[:], masked[:])
    i8 = small.tile([P, 8], U32)
    nc.vector.max_index(i8[:], v8[:], masked[:])
    nrmax = small.tile([P, 1], F32)
    nc.vector.tensor_scalar_mul(nrmax[:], v8[:, 0:1], -1.0)
    exps = bigt.tile([P, E], F32)
    ssum = small.tile([P, 1], F32)
    nc.scalar.activation(
        exps[:], ps[:], ACTF.Exp, bias=nrmax[:], scale=1.0, accum_out=ssum[:]
    )
    rcp = small.tile([P, 1], F32)
    nc.vector.reciprocal(rcp[:], ssum[:])
    scl = small.tile([P, 1], F32)
    nc.vector.tensor_scalar_mul(scl[:], rcp[:], ROUTED_SCALING)
    e6 = small.tile([P, TOP_K], F32)
    nc.scalar.activation(e6[:], v8[:, 0:TOP_K], ACTF.Exp, bias=nrmax[:], scale=1.0)
    w6 = small.tile([P, TOP_K], F32)
    nc.vector.tensor_scalar_mul(w6[:], e6[:], scl[:])
    nc.sync.dma_start(oi_ap[tt * P : (tt + 1) * P, :], i8[:, 0:TOP_K])
    nc.sync.dma_start(ow_ap[tt * P : (tt + 1) * P, :], w6[:])


def build_gate_kernel(T: int = T_CORE, repeat: int = 1, mode: str = "fp32"):
    nc = bacc.Bacc("TRN2", target_bir_lowering=False, debug=False, num_devices=N_CORES)
    oi_d = nc.dram_tensor("oi", [T, TOP_K], U32, kind="ExternalOutput")
    ow_d = nc.dram_tensor("ow", [T, TOP_K], F32, kind="ExternalOutput")
    if mode in ("hilo", "hilo4"):
        x_d = nc.dram_tensor("x", [T, H], F32, kind="ExternalInput")
        whi_d = nc.dram_tensor("whi", [P, J * E], BF16, kind="ExternalInput")
        wlo_d = nc.dram_tensor("wlo", [P, J * E], BF16, kind="ExternalInput")
        with TileContext(nc) as tc:
            for _ in range(repeat):
                emit_gate_hilo(
                    tc, x_d.ap(), whi_d.ap(), wlo_d.ap(), oi_d.ap(), ow_d.ap(),
                    terms=4 if mode == "hilo4" else 3,
                )
    else:
        x_d = nc.dram_tensor("x", [T, H], F32, kind="ExternalInput")
        w_d = nc.dram_tensor("w", [P, J * E], F32, kind="ExternalInput")
        with TileContext(nc) as tc:
            for _ in range(repeat):
                emit_gate(tc, x_d.ap(), w_d.ap(), oi_d.ap(), ow_d.ap())
    nc.compile()
    return nc


def prep_weight(weight: np.ndarray) -> np.ndarray:
    """[160, 5120] -> [128, 40*160] with w[p, j*E + e] = W[e, p*40 + j]."""
    wt = np.asarray(weight, dtype=np.float32).T  # [H, E]
    return np.ascontiguousarray(wt.reshape(P, J, E)).reshape(P, J * E)


def prep_weight_hilo(weight: np.ndarray):
    import ml_dtypes

    w = np.asarray(weight, dtype=np.float32)
    whi = w.astype(ml_dtypes.bfloat16)
    wlo = (w - whi.astype(np.float32)).astype(ml_dtypes.bfloat16)

    def perm(a):
        return np.ascontiguousarray(a.T.reshape(P, J, E)).reshape(P, J * E)

    return perm(whi), perm(wlo)


_NC_CACHE = {}


MODE = "fp32"


def make_in_maps(hidden_states, weight, mode=None):
    mode = mode or MODE
    hs = np.ascontiguousarray(
        np.asarray(hidden_states, dtype=np.float32).reshape(T_TOTAL, H)
    )
    shards = hs.reshape(N_CORES, T_CORE, H)
    if mode in ("hilo", "hilo4"):
        whi, wlo = prep_weight_hilo(weight)
        return [
            {"x": shards[c], "whi": whi, "wlo": wlo} for c in range(N_CORES)
        ]
    wr = prep_weight(weight)
    return [{"x": shards[c], "w": wr} for c in range(N_CORES)]


def run(hidden_states, weight, trace=False, mode=None):
    mode = mode or MODE
    in_maps = make_in_maps(hidden_states, weight, mode)
    if mode not in _NC_CACHE:
        _NC_CACHE[mode] = build_gate_kernel(mode=mode)
    nc = _NC_CACHE[mode]
    res = bass_utils.run_bass_kernel_spmd(
        nc, in_maps, core_ids=list(range(N_CORES)), trace=trace
    )
    idx = np.concatenate([r["oi"].astype(np.int32) for r in res.results], axis=0)
    wts = np.concatenate([r["ow"] for r in res.results], axis=0)
    return (idx, wts), res


def kernel(hidden_states, weight):
    (idx, wts), _ = run(hidden_states, weight)
    return idx, wts
